# revision 7
# baseline (speedup 1.0000x reference)
"""BoundaryLoss kernel for 8 Trainium2 NeuronCores: hybrid gather.

Computes mean_i relu(MARGIN - inputs[i, labels[i]]) over [65536, 1024] f32
inputs, data parallel across 8 cores (8192 rows per core).

The two working gather strategies bottleneck on different engines:
  - indirect_dma_start (hard HW cap: 128 offsets/instruction, one per
    partition): Pool-engine serial, ~1.3us per instruction.
  - full-read + fused select (scalar_tensor_tensor): ~1.3us DVE per
    [128, 1024] tile plus ~1.4us DMA per tile.
Split the 64 row-tiles between the two paths so Pool, DVE and the DMA
engines all run concurrently.  x-tile loads alternate between the SP and
ACT HWDGE queues to pipeline issue.
"""

import os
import sys

for _p in ("/opt/trn_rl_repo", os.path.expanduser("~/.axon_site/_ro/trn_rl_repo")):
    if os.path.isdir(_p) and _p not in sys.path:
        sys.path.insert(0, _p)

import numpy as np

import concourse.bacc as bacc
import concourse.bass as bass
import concourse.mybir as mybir
import concourse.tile as tile
from concourse import bass_utils

POSITIVE_MARGIN = 0.99999
N, G = 65536, 1024
NCORES = 8
NS = N // NCORES
P = 128
T = NS // P  # 64 row-tiles total

# tiles handled by the full-read path; the rest go to the indirect path
FULL_TILES = int(os.environ.get("FULL_TILES", "34"))
IND_TILES = T - FULL_TILES  # indirect path: rows FULL_TILES*128 .. NS
XBUFS = int(os.environ.get("XBUFS", "8"))


def build_program(full_tiles: int = FULL_TILES):
    f32 = mybir.dt.float32
    i32 = mybir.dt.int32
    ind_tiles = T - full_tiles
    ind_base = full_tiles * P  # first row of the indirect block

    nc = bacc.Bacc(
        "TRN2",
        target_bir_lowering=False,
        debug=False,
        dynamic_dma_scratch_size=32768,
    )
    x_t = nc.dram_tensor("inputs", [NS, G], f32, kind="ExternalInput")
    lab_t = nc.dram_tensor("labf_iota", [P, FULL_TILES + G], f32, kind="ExternalInput")
    off_t = nc.dram_tensor("offs", [P, max(ind_tiles, 1)], i32, kind="ExternalInput")
    out_t = nc.dram_tensor("partials", [P, 1], f32, kind="ExternalOutput")

    with tile.TileContext(nc) as tc:
        with tc.tile_pool(name="pool", bufs=1) as pool, tc.tile_pool(
            name="xbuf", bufs=XBUFS
        ) as xbuf:
            full_vals = pool.tile([P, max(full_tiles, 1)], f32)
            ind_vals = pool.tile([P, max(ind_tiles, 1)], f32)

            # ---------- indirect path (host-computed flat offsets) ----------
            if ind_tiles:
                offs = pool.tile([P, ind_tiles], i32)
                nc.sync.dma_start(out=offs[:], in_=off_t.ap())
                for j in range(ind_tiles):
                    nc.gpsimd.indirect_dma_start(
                        out=ind_vals[:, j : j + 1],
                        out_offset=None,
                        in_=x_t.ap(),
                        in_offset=bass.IndirectOffsetOnAxis(
                            ap=offs[:, j : j + 1], axis=1
                        ),
                    )

            # ---------- full-read path ----------
            if full_tiles:
                # host-precomputed: cols 0..ft = labels as f32, then 0..G ramp
                labio = pool.tile([P, full_tiles + G], f32)
                nc.scalar.dma_start(out=labio[:], in_=lab_t.ap())
                lab_f = labio[:, :full_tiles]
                iota_f = labio[:, full_tiles:]

                for t in range(full_tiles):
                    xt = xbuf.tile([P, G], f32, tag="xt")
                    eng = nc.sync if t % 2 == 0 else nc.scalar
                    eng.dma_start(
                        out=xt[:], in_=x_t.ap()[t * P : (t + 1) * P, :]
                    )
                    dummy = xbuf.tile([P, G], f32, tag="dummy")
                    nc.vector.scalar_tensor_tensor(
                        out=dummy[:],
                        in0=iota_f,
                        scalar=lab_f[:, t : t + 1] if False else labio[:, t : t + 1],
                        in1=xt[:],
                        op0=mybir.AluOpType.is_equal,
                        op1=mybir.AluOpType.mult,
                        accum_out=full_vals[:, t : t + 1],
                    )

            # ---------- combine ----------
            clamp_t = pool.tile([P, T], f32)
            nc.vector.tensor_scalar(
                out=clamp_t[:, :full_tiles],
                in0=full_vals[:],
                scalar1=POSITIVE_MARGIN,
                scalar2=0.0,
                op0=mybir.AluOpType.subtract,
                op1=mybir.AluOpType.min,
            )
            nc.vector.tensor_scalar(
                out=clamp_t[:, full_tiles:],
                in0=ind_vals[:],
                scalar1=POSITIVE_MARGIN,
                scalar2=0.0,
                op0=mybir.AluOpType.subtract,
                op1=mybir.AluOpType.min,
            )
            acc = pool.tile([P, 1], f32)
            nc.vector.reduce_sum(acc[:], clamp_t[:], axis=mybir.AxisListType.X)
            nc.scalar.dma_start(out=out_t.ap(), in_=acc[:])

    nc.compile()
    return nc


_PROG = None


def _get_prog():
    global _PROG
    if _PROG is None:
        _PROG = build_program()
    return _PROG


def _make_in_maps(inputs: np.ndarray, labels: np.ndarray):
    inputs = np.asarray(inputs)
    labels = np.asarray(labels)
    assert inputs.shape == (N, G), inputs.shape
    assert labels.shape == (N,), labels.shape
    inputs = np.ascontiguousarray(inputs, dtype=np.float32)

    labi = labels.astype(np.int64)
    ind_base = FULL_TILES * P
    it = IND_TILES
    in_maps = []
    for c in range(NCORES):
        sl = slice(c * NS, (c + 1) * NS)
        ls = labi[c * NS : (c + 1) * NS]
        # offs[p, j] = flat element index of row ind_base + p*it + j
        p = np.arange(P)[:, None]
        j = np.arange(max(it, 1))[None, :]
        rows = ind_base + p * it + j
        offs = (rows * G + ls[np.minimum(rows, NS - 1)]).astype(np.int32)
        # labf_iota: [128, FULL_TILES + G]: lab_f[p, t] = label[t*128+p]; then ramp
        lfi = np.empty((P, FULL_TILES + G), dtype=np.float32)
        t = np.arange(FULL_TILES)[None, :]
        pp = np.arange(P)[:, None]
        lfi[:, :FULL_TILES] = ls[t * P + pp].astype(np.float32)
        lfi[:, FULL_TILES:] = np.arange(G, dtype=np.float32)[None, :]
        in_maps.append(
            {"inputs": inputs[sl], "labf_iota": np.ascontiguousarray(lfi),
             "offs": np.ascontiguousarray(offs)}
        )
    return in_maps


def _run(inputs, labels, trace: bool = False):
    nc = _get_prog()
    in_maps = _make_in_maps(inputs, labels)
    res = bass_utils.run_bass_kernel_spmd(
        nc, in_maps, core_ids=list(range(NCORES)), trace=trace
    )
    total = 0.0
    for r in res.results:
        total += float(np.asarray(r["partials"], dtype=np.float64).sum())
    out = np.array(-total / N, dtype=np.float32)
    return out, res


def kernel(inputs, labels):
    out, _ = _run(inputs, labels, trace=False)
    return out
